# revision 1
# baseline (speedup 1.0000x reference)
"""Trainium2 Bass kernel: DeformableValueAttention (head-parallel rewrite).

Full-input contract: kernel(**inputs) takes the unsharded inputs of
reference.setup_inputs() and returns the full [B, C, H, W] output.

Sharding: 8 cores = (batch b, head-group g). Each core computes 4 of the 8
attention heads for ALL 1024 queries of one batch and produces a PARTIAL
[C, N] output (its 4 heads' contribution through Wo); the host sums the two
partials per batch. Unlike a query-split, nothing is computed twice:
KT / V / Vd / S / O / Wo all shrink to the head group.

Per-core algorithm (channels-on-partitions layouts; all matmuls bf16 with
fp32 PSUM accumulation):
  QT = (Wq_g/8) @ xq        [256, N]   2 head-pair partition-tiles
  KT = Wk_g @ xkv           [256, N]
  V  = xkv^T @ Wv_g^T       [N, 256]   keys on partitions
  Vd = G^T.T @ V            [N, 4*65]  grid_sample as banded sparse matmul;
                                       (1+gamma*sal) folded into G on host;
                                       per head a 65th column of ones
  per head-pair hp, key-tile m, query-half qf:
    S^T = KT^T @ QT         [128, 1024] the two heads ride disjoint PE
                                       row-groups (K=64 each) and overlap
    pu  = exp(S^T)          bf16       no max-subtraction: |scores| < ~6
    O_h += Vd_h^T @ pu_h    [65, N]    ones-column gives the softmax
                                       denominator in row 64 for free
  per head: rec = DVE reciprocal(row 64); GPSIMD partition-broadcast;
    o = O[0:64] * bcast(rec)           written into a [128, N] head-pair
                                       stack (bf16)
  out^T += Wo_g @ o_stack   [C, N]     K=128 per head-pair, fp32 out

The scalar engine runs ONLY Exp (32 activations, one table load); the
softmax reciprocal is DVE InstReciprocal, so there is no Ln/Exp table
ping-pong. The S->exp->O stream is PSUM-double-buffered; projections for
the second head-pair and V/Vd backfill the tensor engine while the ACT
engine drains the first head-pair's exponentials.

PSUM (8 banks): tag "big" 2 slots x 2 banks (projections -> O accumulators
-> Wo, time-shared) + tag "ps_s" 2 slots x 2 banks (scores) = 8.

Notes on fidelity vs reference.py:
  - P_thermal adds a per-query constant to scores pre-softmax; softmax is
    exactly invariant to that, so it is skipped.
  - All biases in setup_inputs() are zeros; nonzero biases or off-spec
    shapes fall back to a numpy reference implementation.
"""

import sys

import numpy as np
import ml_dtypes

try:
    import concourse.bass as bass  # noqa: F401
except ImportError:  # pragma: no cover - path fallback for bare containers
    sys.path.insert(0, "/opt/trn_rl_repo")
    import concourse.bass as bass  # noqa: F401

import concourse.bacc as bacc
import concourse.tile as tile
from concourse import mybir
from concourse.bass_utils import run_bass_kernel_spmd

B, C, HH, WW = 4, 512, 32, 32
N = HH * WW          # 1024 spatial positions = keys = queries
NH, HD = 8, 64       # total heads, head dim
G = 2                # head groups (cores per batch)
HG = NH // G         # heads per core (4)
CG = HG * HD         # channels per core (256)
P = 128
CT = C // P          # 4 input-channel partition-tiles
NKT = N // P         # 8 key tiles
NQF = N // 512       # 2 query free-chunks per matmul
NCORES = 8
BF16 = mybir.dt.bfloat16
FP32 = mybir.dt.float32
NP_BF16 = ml_dtypes.bfloat16


# --------------------------------------------------------------------------
# host-side helpers
# --------------------------------------------------------------------------

def _gather_T(offsets_b, salf_b):
    """GT[k, n]: weight of source pixel k in grid-sampled output pixel n,
    with the per-source value modulation salf folded in. fp32 [N, N]."""
    ys = np.linspace(-1.0, 1.0, HH)
    xs = np.linspace(-1.0, 1.0, WW)
    gy, gx = np.meshgrid(ys, xs, indexing="ij")
    x = ((gx + offsets_b[0] / (WW / 2.0) + 1.0) * WW - 1.0) * 0.5
    y = ((gy + offsets_b[1] / (HH / 2.0) + 1.0) * HH - 1.0) * 0.5
    x = np.clip(x, 0.0, WW - 1.0)
    y = np.clip(y, 0.0, HH - 1.0)
    x0 = np.floor(x); y0 = np.floor(y)
    wx = x - x0; wy = y - y0
    x0i = x0.astype(np.int64); y0i = y0.astype(np.int64)
    x1i = np.minimum(x0i + 1, WW - 1); y1i = np.minimum(y0i + 1, HH - 1)
    GT = np.zeros((N, N), np.float32)
    n_idx = np.arange(N)
    for yi, xi, w in ((y0i, x0i, (1 - wx) * (1 - wy)),
                      (y0i, x1i, wx * (1 - wy)),
                      (y1i, x0i, (1 - wx) * wy),
                      (y1i, x1i, wx * wy)):
        np.add.at(GT, ((yi * WW + xi).reshape(-1), n_idx),
                  w.reshape(-1).astype(np.float32))
    GT *= salf_b[:, None]
    return GT


def _reference_numpy(q_feat, kv_feat, offsets, saliency_map, P_thermal,
                     Wq, bq, Wk, bk, Wv, bv, Wo, bo, lambda_p, gamma_val):
    """Plain numpy port of reference.py -- correctness fallback only."""
    Bq, Cq = q_feat.shape[0], q_feat.shape[1]
    Nq = q_feat.shape[2] * q_feat.shape[3]
    qf = q_feat.reshape(Bq, Cq, Nq).transpose(0, 2, 1)
    kf = kv_feat.reshape(Bq, Cq, Nq).transpose(0, 2, 1)

    def heads(x, Wm, bm):
        return (x @ Wm.T + bm).reshape(Bq, Nq, NH, -1).transpose(0, 2, 1, 3)

    Q = heads(qf, Wq, bq)
    K = heads(kf, Wk, bk)
    V = heads(kf, Wv, bv)
    hd = Cq // NH
    attn = np.einsum("bhqd,bhkd->bhqk", Q, K) * (hd ** -0.5)
    attn = attn + float(lambda_p) * P_thermal.reshape(Bq, 1, Nq, 1)
    attn = attn - attn.max(axis=-1, keepdims=True)
    w = np.exp(attn)
    w /= w.sum(axis=-1, keepdims=True)
    Vm = V * (1.0 + float(gamma_val) * saliency_map.reshape(Bq, 1, Nq, 1))
    Vsp = Vm.transpose(0, 2, 1, 3).reshape(Bq, Nq, Cq).transpose(0, 2, 1)
    Vd = np.empty_like(Vsp)
    for b in range(Bq):
        GT = _gather_T(offsets[b], np.ones(Nq, np.float32))
        Vd[b] = Vsp[b] @ GT
    Vdf = Vd.reshape(Bq, Cq, Nq).transpose(0, 2, 1).reshape(Bq, Nq, NH, hd).transpose(0, 2, 1, 3)
    out = np.einsum("bhqk,bhkd->bhqd", w, Vdf)
    out = out.transpose(0, 2, 1, 3).reshape(Bq, Nq, Cq)
    out = out @ Wo.T + bo
    return out.transpose(0, 2, 1).reshape(q_feat.shape).astype(np.float32)


# --------------------------------------------------------------------------
# device program
# --------------------------------------------------------------------------

def _build_program(chunks):
    """chunks: ordered list of (m, k) gather-tile pairs; same for all cores."""
    nch = len(chunks)
    chunks_for_m = {m: [] for m in range(NKT)}
    for idx, (m, k) in enumerate(chunks):
        chunks_for_m[m].append((idx, k))

    nc = bacc.Bacc(None, target_bir_lowering=False, debug=False)
    xq_d = nc.declare_dram_parameter("xq", [C, N], BF16, isOutput=False)
    xkv_d = nc.declare_dram_parameter("xkv", [C, N], BF16, isOutput=False)
    wq_d = nc.declare_dram_parameter("wqT", [C, CG], BF16, isOutput=False)
    wk_d = nc.declare_dram_parameter("wkT", [C, CG], BF16, isOutput=False)
    wv_d = nc.declare_dram_parameter("wvT", [C, CG], BF16, isOutput=False)
    wo_d = nc.declare_dram_parameter("woT", [CG, C], BF16, isOutput=False)
    gt_d = nc.declare_dram_parameter("gt", [nch, P, P], BF16, isOutput=False)
    out_d = nc.declare_dram_parameter("outT", [C, N], FP32, isOutput=True)

    with tile.TileContext(nc) as tc:
        with tc.tile_pool(name="const", bufs=1) as const, \
             tc.tile_pool(name="work", bufs=1) as work, \
             tc.tile_pool(name="pu_pool", bufs=1) as pu_pool, \
             tc.tile_pool(name="sm", bufs=4) as sm, \
             tc.tile_pool(name="psp", bufs=2, space="PSUM") as psp:

            # ---- input DMAs: the QT/KT first wave is interleaved across all
            # four queues so the first S pair can start ~8us in; wv/gt/wo
            # trail on gpsimd (not needed until the V/Vd backfill).
            def decl(nm, width, k):
                return const.tile([P, width], BF16, name=f"{nm}{k}",
                                  tag=f"{nm}{k}")

            wq_sb = [decl("wq", CG, k) for k in range(CT)]
            xq_sb = [decl("xq", N, k) for k in range(CT)]
            wk_sb = [decl("wk", CG, k) for k in range(CT)]
            xkv_sb = [decl("xkv", N, k) for k in range(CT)]
            wv_sb = [decl("wv", CG, k) for k in range(CT)]

            def ld(engine, tl, dram, k):
                engine.dma_start(out=tl[k][:], in_=dram[k * P:(k + 1) * P, :])

            queues = [nc.sync, nc.scalar, nc.gpsimd]
            for k in range(CT):
                qq = queues[k % 3]
                qk = queues[(k + 1) % 3]
                ld(qq, wq_sb, wq_d, k)
                ld(qq, xq_sb, xq_d, k)
                ld(qk, wk_sb, wk_d, k)
                ld(qk, xkv_sb, xkv_d, k)
            for k in range(CT):
                ld(nc.gpsimd, wv_sb, wv_d, k)
            gt_w = const.tile([P, nch * P], BF16, name="gtw", tag="gtw")
            nc.gpsimd.dma_start(
                out=gt_w[:].rearrange("p (c j) -> p c j", j=P),
                in_=gt_d[:].rearrange("c p j -> p c j"))
            # wo stacked head-pair-major: wo_sb[p, hp*C + c] = woT[hp*128+p, c]
            wo_sb = const.tile([P, G * C], BF16, name="wo", tag="wo")
            nc.sync.dma_start(
                out=wo_sb[:].rearrange("p (hp c) -> p hp c", c=C),
                in_=wo_d[:].rearrange("(hp p) c -> p hp c", p=P))

            qt_sb, kt_sb, v_sb, vd_sb = {}, {}, {}, {}
            pu_tiles = {}
            ps_o = {}
            o_sb = {}

            # ---- emission helpers -----------------------------------------
            def emit_qtkt(pt):
                # k-outer, QT/KT interleaved: each arriving DMA tile feeds
                # 2+2 matmuls immediately; one [P, N] PSUM tile per dst with
                # per-qf accumulation regions.
                dq = work.tile([P, N], BF16, name=f"qt{pt}", tag=f"qt{pt}")
                dk = work.tile([P, N], BF16, name=f"kt{pt}", tag=f"kt{pt}")
                psq = psp.tile([P, N], FP32, name=f"pspq{pt}", tag="big",
                               bufs=2)
                psk = psp.tile([P, N], FP32, name=f"pspk{pt}", tag="big",
                               bufs=2)
                for qf in range(NQF):
                    for k in range(CT):
                        nc.tensor.matmul(
                            psq[:, qf * 512:(qf + 1) * 512],
                            lhsT=wq_sb[k][:, pt * P:(pt + 1) * P],
                            rhs=xq_sb[k][:, qf * 512:(qf + 1) * 512],
                            start=(k == 0), stop=(k == CT - 1))
                for qf in range(NQF):
                    for k in range(CT):
                        nc.tensor.matmul(
                            psk[:, qf * 512:(qf + 1) * 512],
                            lhsT=wk_sb[k][:, pt * P:(pt + 1) * P],
                            rhs=xkv_sb[k][:, qf * 512:(qf + 1) * 512],
                            start=(k == 0), stop=(k == CT - 1))
                nc.vector.tensor_copy(dq[:], psq[:])
                nc.vector.tensor_copy(dk[:], psk[:])
                qt_sb[pt], kt_sb[pt] = dq, dk

            def emit_v(m):
                ps = psp.tile([P, N], FP32, name=f"psv{m}", tag="big", bufs=2)
                for k in range(CT):
                    nc.tensor.matmul(ps[:, 0:CG],
                                     lhsT=xkv_sb[k][:, m * P:(m + 1) * P],
                                     rhs=wv_sb[k][:],
                                     start=(k == 0), stop=(k == CT - 1))
                tl = work.tile([P, CG], BF16, name=f"v{m}", tag=f"v{m}")
                nc.vector.tensor_copy(tl[:], ps[:, 0:CG])
                v_sb[m] = tl

            def emit_vd(m):
                ps = psp.tile([P, N], FP32, name=f"psvd{m}", tag="big", bufs=2)
                lst = chunks_for_m[m]
                for j, (idx, k) in enumerate(lst):
                    nc.tensor.matmul(ps[:, 0:CG],
                                     lhsT=gt_w[:, idx * P:(idx + 1) * P],
                                     rhs=v_sb[k][:],
                                     start=(j == 0), stop=(j == len(lst) - 1))
                tl = work.tile([P, HG * (HD + 1)], BF16,
                               name=f"vd{m}", tag=f"vd{m}")
                tl3 = tl[:].rearrange("p (h e) -> p h e", e=HD + 1)
                nc.vector.tensor_copy(
                    tl3[:, :, 0:HD],
                    ps[:, 0:CG].rearrange("p (h e) -> p h e", e=HD))
                nc.vector.memset(tl3[:, :, HD:HD + 1], 1.0)
                vd_sb[m] = tl

            def emit_s(hp, m):
                # scores for both heads of pair hp, key-tile m, both query
                # halves; exp straight off PSUM into a bf16 pu tile.
                kt, qt = kt_sb[hp], qt_sb[hp]
                for qf in range(NQF):
                    ps_s = psp.tile([P, N], FP32, name=f"pss{hp}{m}{qf}",
                                    tag="ps_s", bufs=2)
                    nc.tensor.matmul(
                        ps_s[:, 0:512],
                        lhsT=kt[0:HD, m * P:(m + 1) * P],
                        rhs=qt[0:HD, qf * 512:(qf + 1) * 512],
                        start=True, stop=True)
                    nc.tensor.matmul(
                        ps_s[:, 512:1024],
                        lhsT=kt[HD:P, m * P:(m + 1) * P],
                        rhs=qt[HD:P, qf * 512:(qf + 1) * 512],
                        start=True, stop=True)
                    pu = pu_pool.tile([P, N], BF16, name=f"pu{hp}{m}{qf}",
                                      tag="pu", bufs=16)
                    nc.scalar.activation(out=pu[:], in_=ps_s[:],
                                         func=mybir.ActivationFunctionType.Exp)
                    pu_tiles[(hp, m, qf)] = pu

            def emit_o(hp, m, qfs=(0, 1)):
                if (hp, 0) not in ps_o:
                    ps_o[(hp, 0)] = psp.tile([HD + 1, N], FP32,
                                             name=f"pso{hp}a", tag="big",
                                             bufs=2)
                    ps_o[(hp, 1)] = psp.tile([HD + 1, N], FP32,
                                             name=f"pso{hp}b", tag="big",
                                             bufs=2)
                vd3 = vd_sb[m][:].rearrange("p (h e) -> p h e", e=HD + 1)
                for qf in qfs:
                    pu = pu_tiles[(hp, m, qf)]
                    for hh in range(2):
                        nc.tensor.matmul(
                            ps_o[(hp, hh)][:, qf * 512:(qf + 1) * 512],
                            lhsT=vd3[:, 2 * hp + hh, :],
                            rhs=pu[:, hh * 512:(hh + 1) * 512],
                            start=(m == 0), stop=(m == NKT - 1))

            def emit_norm(hp, qfs=(0, 1)):
                # o_sb[hp][0:64] = head 2hp, [64:128] = head 2hp+1 (bf16),
                # each row block scaled by its softmax reciprocal
                # (approx-fast DVE recip + GPSIMD partition broadcast).
                if hp in o_sb:
                    ot = o_sb[hp]
                else:
                    ot = work.tile([P, N], BF16, name=f"o{hp}", tag=f"o{hp}")
                    o_sb[hp] = ot
                for qf in qfs:
                    cols = slice(qf * 512, (qf + 1) * 512)
                    recs = []
                    for hh in range(2):
                        dn = sm.tile([1, 512], FP32, name=f"dn{hp}{hh}{qf}",
                                     tag="dn", bufs=4)
                        nc.vector.tensor_copy(
                            dn[:], ps_o[(hp, hh)][HD:HD + 1, cols])
                        rec = sm.tile([1, 512], FP32, name=f"rec{hp}{hh}{qf}",
                                      tag="rec", bufs=4)
                        nc.vector.reciprocal_approx_fast(rec[:], dn[:])
                        recs.append(rec)
                    for hh in range(2):
                        bc = sm.tile([HD, 512], FP32, name=f"bc{hp}{hh}{qf}",
                                     tag="bc", bufs=2)
                        nc.gpsimd.partition_broadcast(bc[:], recs[hh][:])
                        nc.vector.tensor_mul(ot[hh * HD:(hh + 1) * HD, cols],
                                             ps_o[(hp, hh)][0:HD, cols],
                                             bc[:])

            def emit_wo(qf):
                cols = slice(qf * 512, (qf + 1) * 512)
                for pt in range(CT):
                    ps = psp.tile([P, N], FP32, name=f"psw{pt}{qf}",
                                  tag="ps_s", bufs=2)
                    for hp in range(G):
                        nc.tensor.matmul(
                            ps[:, cols],
                            lhsT=wo_sb[:, hp * C + pt * P:
                                       hp * C + (pt + 1) * P],
                            rhs=o_sb[hp][:, cols],
                            start=(hp == 0), stop=(hp == G - 1))
                    ob = sm.tile([P, 512], FP32, name=f"ob{pt}{qf}",
                                 tag="ob", bufs=2)
                    nc.vector.tensor_copy(ob[:], ps[:, cols])
                    nc.sync.dma_start(out=out_d[pt * P:(pt + 1) * P, cols],
                                      in_=ob[:])

            # ---- emission schedule ----------------------------------------
            emit_qtkt(0)
            # hp0 S/exp stream with projection backfill on the tensor queue
            backfill = ([lambda m=m: emit_v(m) for m in range(NKT)]
                        + [lambda: emit_qtkt(1)]
                        + [lambda m=m: emit_vd(m) for m in range(NKT)])
            bi = 0
            steps = [2, 2, 2, 2, 1, 2, 3, 3]  # backfill items after S(m)
            for m in range(NKT):
                emit_s(0, m)
                for _ in range(steps[m]):
                    if bi < len(backfill):
                        backfill[bi]()
                        bi += 1
            while bi < len(backfill):
                backfill[bi]()
                bi += 1
            # hp0 O (front-loaded, so norm(0) hides under hp1's S stream)
            # interleaved with hp1 S/exp
            for m in range(4):
                emit_o(0, 2 * m)
                emit_o(0, 2 * m + 1)
                emit_s(1, m)
            emit_norm(0)
            # remaining hp1 S with early hp1 O backfill (needs norm(0) to
            # have released the hp0 accumulator PSUM slots)
            for m in range(4, NKT):
                emit_s(1, m)
                emit_o(1, m - 4)
            # tail, split by query half: norm/Wo of qf0 hide under qf1's O
            for m in range(4, NKT):
                emit_o(1, m, qfs=(0,))
            emit_norm(1, qfs=(0,))
            for m in range(4, NKT):
                emit_o(1, m, qfs=(1,))
            emit_wo(0)
            emit_norm(1, qfs=(1,))
            emit_wo(1)

    nc.compile()
    return nc


# --------------------------------------------------------------------------
# public entry points
# --------------------------------------------------------------------------

def _prepare(inputs):
    q = np.ascontiguousarray(inputs["q_feat"], np.float32).reshape(B, C, N)
    kv = np.ascontiguousarray(inputs["kv_feat"], np.float32).reshape(B, C, N)
    offsets = np.asarray(inputs["offsets"], np.float32)
    sal = np.asarray(inputs["saliency_map"], np.float32).reshape(B, N)
    gamma = float(np.asarray(inputs["gamma_val"]))

    GTs = [_gather_T(offsets[b], 1.0 + gamma * sal[b]) for b in range(B)]

    # union band-sparsity pattern of the gather matmul across batches, so the
    # SPMD program is identical on every core
    chunks = []
    for m in range(NKT):
        for k in range(NKT):
            if any(GTs[b][k * P:(k + 1) * P, m * P:(m + 1) * P].any()
                   for b in range(B)):
                chunks.append((m, k))

    Wq = np.asarray(inputs["Wq"], np.float32) * (HD ** -0.5)
    Wk = np.asarray(inputs["Wk"], np.float32)
    Wv = np.asarray(inputs["Wv"], np.float32)
    Wo = np.asarray(inputs["Wo"], np.float32)

    in_maps = []
    for core in range(NCORES):
        b, g = core // G, core % G
        rows = slice(g * CG, (g + 1) * CG)
        gt_stack = np.stack([
            np.ascontiguousarray(
                GTs[b][k * P:(k + 1) * P, m * P:(m + 1) * P]).astype(NP_BF16)
            for (m, k) in chunks])
        in_maps.append({
            "xq": np.ascontiguousarray(q[b]).astype(NP_BF16),
            "xkv": np.ascontiguousarray(kv[b]).astype(NP_BF16),
            "wqT": np.ascontiguousarray(Wq[rows].T).astype(NP_BF16),
            "wkT": np.ascontiguousarray(Wk[rows].T).astype(NP_BF16),
            "wvT": np.ascontiguousarray(Wv[rows].T).astype(NP_BF16),
            "woT": np.ascontiguousarray(Wo[:, rows].T).astype(NP_BF16),
            "gt": gt_stack,
        })

    def assemble(results):
        out = np.empty((B, C, N), np.float32)
        for b in range(B):
            out[b] = results[G * b]["outT"] + results[G * b + 1]["outT"]
        return out.reshape(B, C, HH, WW)

    nc = _build_program(chunks)
    return nc, in_maps, assemble


def _needs_fallback(inputs):
    try:
        if tuple(np.shape(inputs["q_feat"])) != (B, C, HH, WW):
            return True
        for bias in ("bq", "bk", "bv", "bo"):
            if np.any(np.asarray(inputs[bias], np.float32) != 0.0):
                return True
    except Exception:
        return True
    return False


def kernel(**inputs) -> np.ndarray:
    if _needs_fallback(inputs):
        return _reference_numpy(**{k: np.asarray(v, np.float32)
                                   for k, v in inputs.items()})
    nc, in_maps, assemble = _prepare(inputs)
    res = run_bass_kernel_spmd(nc, in_maps, core_ids=list(range(NCORES)))
    return assemble(res.results)


def kernel_traced(trace_cores=(0,), **inputs):
    """Like kernel() but returns (output, exec_time_ns, trace_path)."""
    nc, in_maps, assemble = _prepare(inputs)
    res = run_bass_kernel_spmd(nc, in_maps, core_ids=list(range(NCORES)),
                               trace=True, trace_cores=list(trace_cores))
    trace_path = None
    if res.instructions_and_trace is not None:
        trace_path = res.instructions_and_trace[1]
    return assemble(res.results), res.exec_time_ns, trace_path



# revision 8
# speedup vs baseline: 1.3111x; 1.3111x over previous
"""Trainium2 Bass kernel: DeformableValueAttention (exp-spine schedule, v2).

Full-input contract: kernel(**inputs) takes the unsharded inputs of
reference.setup_inputs() and returns the full [B, C, H, W] output.

Sharding: 8 cores = (batch b, head-group g). Each core computes 4 of the 8
attention heads for ALL 1024 queries of one batch and produces a PARTIAL
[C, N] output (its 4 heads' contribution through Wo, fp16); the host sums
the two partials per batch in fp32.

v2 schedule ("exp spine"): the ACT engine's 32 Exp tiles (~1.07us each on
[128,1024] fp32 PSUM -> bf16) are the hard serial resource (~34us); the PE
work (~36us at 2.4GHz) is arranged so the spine never stalls:

  A (t~2.5-5us)  QT(hp0,qf0) + KT(hp0) chunk0 as soon as their DMA lands.
  B (spine hp0)  16 units: S-pair(hp0,m,qf) -> exp. PE backfill between
                 pairs: KT(hp0) JIT chunks, QT(hp0,qf1), QT/KT(hp1), V, Vd.
  C (spine hp1)  qf-outer: 8 units (qf0,m0..7), then 8 (qf1,m0..7).
                 PE backfill: O(hp0) (all m, qf-grouped) -> norm(hp0) ->
                 O(hp1) lag-behind; norm(hp1,qf0) closes mid-C.
  D (tail ~5us)  last O pair -> Wo(qf0) || norm(hp1,qf1) -> Wo(qf1),
                 per-pt pipelined with fp16 copies + out DMA.

PSUM (8 banks): tag "ps_s" 2 x [128,1024] (4 banks) for scores; tag
"ps_acc" 4 x [<=128,<=512] (4 banks) time-shared: projections/V/Vd (B) ->
O accumulators of one head-pair at a time (C) -> Wo chunks (D).

Engines: PE matmuls; ACT only Exp (one table load); DVE reciprocal (read
straight from the accumulator's ones-row in PSUM), norm multiply, output
fp16 copies, Vd copies; Pool partition-broadcasts, QT/KT/V copies, and the
xkv/gt DMA queue. Input DMAs are priority-ordered and chunked so the first
S pair fires ~4us in.

Notes on fidelity vs reference.py:
  - P_thermal adds a per-query constant to scores pre-softmax; softmax is
    exactly invariant to that, so it is skipped.
  - All biases in setup_inputs() are zeros; nonzero biases or off-spec
    shapes fall back to a numpy reference implementation.
"""

import sys

import numpy as np
import ml_dtypes

try:
    import concourse.bass as bass  # noqa: F401
except ImportError:  # pragma: no cover - path fallback for bare containers
    sys.path.insert(0, "/opt/trn_rl_repo")
    import concourse.bass as bass  # noqa: F401

import concourse.bacc as bacc
import concourse.tile as tile
from concourse import mybir
from concourse.bass_utils import run_bass_kernel_spmd

B, C, HH, WW = 4, 512, 32, 32
N = HH * WW          # 1024 spatial positions = keys = queries
NH, HD = 8, 64       # total heads, head dim
G = 2                # head groups (cores per batch)
HG = NH // G         # heads per core (4)
CG = HG * HD         # channels per core (256)
P = 128
CT = C // P          # 4 input-channel partition-tiles
NKT = N // P         # 8 key tiles
NCORES = 8
BF16 = mybir.dt.bfloat16
FP16 = mybir.dt.float16
FP32 = mybir.dt.float32
NP_BF16 = ml_dtypes.bfloat16
EXP = mybir.ActivationFunctionType.Exp


# --------------------------------------------------------------------------
# host-side helpers
# --------------------------------------------------------------------------

def _gather_T(offsets_b, salf_b):
    """GT[k, n]: weight of source pixel k in grid-sampled output pixel n,
    with the per-source value modulation salf folded in. fp32 [N, N]."""
    ys = np.linspace(-1.0, 1.0, HH)
    xs = np.linspace(-1.0, 1.0, WW)
    gy, gx = np.meshgrid(ys, xs, indexing="ij")
    x = ((gx + offsets_b[0] / (WW / 2.0) + 1.0) * WW - 1.0) * 0.5
    y = ((gy + offsets_b[1] / (HH / 2.0) + 1.0) * HH - 1.0) * 0.5
    x = np.clip(x, 0.0, WW - 1.0)
    y = np.clip(y, 0.0, HH - 1.0)
    x0 = np.floor(x); y0 = np.floor(y)
    wx = x - x0; wy = y - y0
    x0i = x0.astype(np.int64); y0i = y0.astype(np.int64)
    x1i = np.minimum(x0i + 1, WW - 1); y1i = np.minimum(y0i + 1, HH - 1)
    GT = np.zeros((N, N), np.float32)
    n_idx = np.arange(N)
    for yi, xi, w in ((y0i, x0i, (1 - wx) * (1 - wy)),
                      (y0i, x1i, wx * (1 - wy)),
                      (y1i, x0i, (1 - wx) * wy),
                      (y1i, x1i, wx * wy)):
        np.add.at(GT, ((yi * WW + xi).reshape(-1), n_idx),
                  w.reshape(-1).astype(np.float32))
    GT *= salf_b[:, None]
    return GT


def _reference_numpy(q_feat, kv_feat, offsets, saliency_map, P_thermal,
                     Wq, bq, Wk, bk, Wv, bv, Wo, bo, lambda_p, gamma_val):
    """Plain numpy port of reference.py -- correctness fallback only."""
    Bq, Cq = q_feat.shape[0], q_feat.shape[1]
    Nq = q_feat.shape[2] * q_feat.shape[3]
    qf = q_feat.reshape(Bq, Cq, Nq).transpose(0, 2, 1)
    kf = kv_feat.reshape(Bq, Cq, Nq).transpose(0, 2, 1)

    def heads(x, Wm, bm):
        return (x @ Wm.T + bm).reshape(Bq, Nq, NH, -1).transpose(0, 2, 1, 3)

    Q = heads(qf, Wq, bq)
    K = heads(kf, Wk, bk)
    V = heads(kf, Wv, bv)
    hd = Cq // NH
    attn = np.einsum("bhqd,bhkd->bhqk", Q, K) * (hd ** -0.5)
    attn = attn + float(lambda_p) * P_thermal.reshape(Bq, 1, Nq, 1)
    attn = attn - attn.max(axis=-1, keepdims=True)
    w = np.exp(attn)
    w /= w.sum(axis=-1, keepdims=True)
    Vm = V * (1.0 + float(gamma_val) * saliency_map.reshape(Bq, 1, Nq, 1))
    Vsp = Vm.transpose(0, 2, 1, 3).reshape(Bq, Nq, Cq).transpose(0, 2, 1)
    Vd = np.empty_like(Vsp)
    for b in range(Bq):
        GT = _gather_T(offsets[b], np.ones(Nq, np.float32))
        Vd[b] = Vsp[b] @ GT
    Vdf = Vd.reshape(Bq, Cq, Nq).transpose(0, 2, 1).reshape(Bq, Nq, NH, hd).transpose(0, 2, 1, 3)
    out = np.einsum("bhqk,bhkd->bhqd", w, Vdf)
    out = out.transpose(0, 2, 1, 3).reshape(Bq, Nq, Cq)
    out = out @ Wo.T + bo
    return out.transpose(0, 2, 1).reshape(q_feat.shape).astype(np.float32)


# --------------------------------------------------------------------------
# device program
# --------------------------------------------------------------------------

def _build_program(chunks):
    """chunks: ordered list of (m, k) gather-tile pairs; same for all cores."""
    nch = len(chunks)
    chunks_for_m = {m: [] for m in range(NKT)}
    for idx, (m, k) in enumerate(chunks):
        chunks_for_m[m].append((idx, k))

    nc = bacc.Bacc(None, target_bir_lowering=False, debug=False)
    xq_d = nc.declare_dram_parameter("xq", [P, CT * N], BF16, isOutput=False)
    xkv_d = nc.declare_dram_parameter("xkv", [P, CT * N], BF16,
                                      isOutput=False)
    wq_d = nc.declare_dram_parameter("wqT", [P, CT * CG], BF16,
                                     isOutput=False)
    wk_d = nc.declare_dram_parameter("wkT", [P, CT * CG], BF16,
                                     isOutput=False)
    wv_d = nc.declare_dram_parameter("wvT", [P, CT * CG], BF16,
                                     isOutput=False)
    wo_d = nc.declare_dram_parameter("woT", [P, G * C], BF16, isOutput=False)
    gt_d = nc.declare_dram_parameter("gt", [P, nch * P], BF16,
                                     isOutput=False)
    out_d = nc.declare_dram_parameter("outT", [C, N], FP16, isOutput=True)

    with tile.TileContext(nc) as tc:
        with tc.tile_pool(name="const", bufs=1) as const, \
             tc.tile_pool(name="work", bufs=1) as work, \
             tc.tile_pool(name="pu_pool", bufs=1) as pu_pool, \
             tc.tile_pool(name="sm", bufs=4) as sm, \
             tc.tile_pool(name="psp", bufs=2, space="PSUM") as psp:

            # ---- SBUF input tiles (host pre-tiled to [128, X] layouts) ----
            xq_sb = const.tile([P, CT * N], BF16, name="xq", tag="xq")
            xkv_sb = const.tile([P, CT * N], BF16, name="xkv", tag="xkv")
            wq_sb = const.tile([P, CT * CG], BF16, name="wq", tag="wq")
            wk_sb = const.tile([P, CT * CG], BF16, name="wk", tag="wk")
            wv_sb = const.tile([P, CT * CG], BF16, name="wv", tag="wv")
            gt_w = const.tile([P, nch * P], BF16, name="gtw", tag="gtw")
            wo_sb = const.tile([P, G * C], BF16, name="wo", tag="wo")

            # ---- input DMAs, priority-ordered per queue -------------------
            # sync: xq qf0 then qf1 (QT(hp0,qf0) starts ~3us in); scalar: the
            # weight stack (ACT is idle until the first exp); pool: xkv in
            # 256-col chunks (KT m-chunks go just-in-time), then gt.
            def colchunk(t, lo, hi):
                return t[:].rearrange("p (k n) -> p k n", n=N)[:, :, lo:hi]

            nc.sync.dma_start(out=colchunk(xq_sb, 0, 512),
                              in_=colchunk(xq_d, 0, 512))
            nc.sync.dma_start(out=colchunk(xq_sb, 512, 1024),
                              in_=colchunk(xq_d, 512, 1024))
            for w_d, w_sb in ((wq_d, wq_sb), (wk_d, wk_sb), (wv_d, wv_sb),
                              (wo_d, wo_sb)):
                nc.scalar.dma_start(out=w_sb[:], in_=w_d[:])
            for j in range(4):
                nc.gpsimd.dma_start(
                    out=colchunk(xkv_sb, j * 256, (j + 1) * 256),
                    in_=colchunk(xkv_d, j * 256, (j + 1) * 256))
            nc.gpsimd.dma_start(out=gt_w[:], in_=gt_d[:])

            # ---- SBUF result tiles ----------------------------------------
            qt_sb = {hp: work.tile([P, N], BF16, name=f"qt{hp}", tag=f"qt{hp}")
                     for hp in range(G)}
            kt_sb = {hp: work.tile([P, N], BF16, name=f"kt{hp}", tag=f"kt{hp}")
                     for hp in range(G)}
            v_sb = {}
            vd_sb = {}
            o_sb = {hp: work.tile([P, N], BF16, name=f"o{hp}", tag=f"o{hp}")
                    for hp in range(G)}
            pu_tiles = {}
            ps_o = {}

            # ---- emission helpers -----------------------------------------
            def emit_qt_chunk(hp, qf):
                # qt_sb[hp][:, qf*512:] = (Wq_hp @ xq)[:, qf half]
                cols = slice(qf * 512, (qf + 1) * 512)
                ps = psp.tile([P, 512], FP32, name=f"psq{hp}{qf}",
                              tag="ps_acc", bufs=4)
                for k in range(CT):
                    nc.tensor.matmul(
                        ps[:],
                        lhsT=wq_sb[:, k * CG + hp * P:k * CG + (hp + 1) * P],
                        rhs=xq_sb[:, k * N + qf * 512:k * N + (qf + 1) * 512],
                        start=(k == 0), stop=(k == CT - 1))
                nc.vector.tensor_copy(qt_sb[hp][:, cols], ps[:])

            def emit_kt_chunk(hp, j):
                # kt_sb[hp][:, j*256:(j+1)*256] (key-tile pair 2j, 2j+1)
                cols = slice(j * 256, (j + 1) * 256)
                ps = psp.tile([P, 256], FP32, name=f"psk{hp}{j}",
                              tag="ps_acc", bufs=4)
                for k in range(CT):
                    nc.tensor.matmul(
                        ps[:],
                        lhsT=wk_sb[:, k * CG + hp * P:k * CG + (hp + 1) * P],
                        rhs=xkv_sb[:, k * N + j * 256:k * N + (j + 1) * 256],
                        start=(k == 0), stop=(k == CT - 1))
                nc.vector.tensor_copy(kt_sb[hp][:, cols], ps[:])

            def emit_v(m):
                ps = psp.tile([P, CG], FP32, name=f"psv{m}", tag="ps_acc",
                              bufs=4)
                for k in range(CT):
                    nc.tensor.matmul(ps[:],
                                     lhsT=xkv_sb[:, k * N + m * P:k * N + (m + 1) * P],
                                     rhs=wv_sb[:, k * CG:(k + 1) * CG],
                                     start=(k == 0), stop=(k == CT - 1))
                tl = work.tile([P, CG], BF16, name=f"v{m}", tag=f"v{m}")
                nc.vector.tensor_copy(tl[:], ps[:])
                v_sb[m] = tl

            def emit_vd(m):
                ps = psp.tile([P, CG], FP32, name=f"psvd{m}", tag="ps_acc",
                              bufs=4)
                lst = chunks_for_m[m]
                for j, (idx, k) in enumerate(lst):
                    nc.tensor.matmul(ps[:],
                                     lhsT=gt_w[:, idx * P:(idx + 1) * P],
                                     rhs=v_sb[k][:],
                                     start=(j == 0), stop=(j == len(lst) - 1))
                tl = work.tile([P, HG * (HD + 1)], BF16,
                               name=f"vd{m}", tag=f"vd{m}")
                tl3 = tl[:].rearrange("p (h e) -> p h e", e=HD + 1)
                nc.vector.tensor_copy(
                    tl3[:, :, 0:HD],
                    ps[:].rearrange("p (h e) -> p h e", e=HD))
                nc.vector.memset(tl3[:, :, HD:HD + 1], 1.0)
                vd_sb[m] = tl

            def emit_s(hp, m, qf):
                # scores for both heads of pair hp, key-tile m, query half
                # qf; exp straight off PSUM into a bf16 pu tile.
                kt, qt = kt_sb[hp], qt_sb[hp]
                ps_s = psp.tile([P, N], FP32, name=f"pss{hp}{m}{qf}",
                                tag="ps_s", bufs=2)
                nc.tensor.matmul(
                    ps_s[:, 0:512],
                    lhsT=kt[0:HD, m * P:(m + 1) * P],
                    rhs=qt[0:HD, qf * 512:(qf + 1) * 512],
                    start=True, stop=True)
                nc.tensor.matmul(
                    ps_s[:, 512:1024],
                    lhsT=kt[HD:P, m * P:(m + 1) * P],
                    rhs=qt[HD:P, qf * 512:(qf + 1) * 512],
                    start=True, stop=True)
                pu = pu_pool.tile([P, N], BF16, name=f"pu{hp}{m}{qf}",
                                  tag="pu", bufs=22)
                nc.scalar.activation(out=pu[:], in_=ps_s[:], func=EXP)
                pu_tiles[(hp, m, qf)] = pu

            def alloc_ps_o(hp, qf):
                for hh in range(2):
                    ps_o[(hp, hh, qf)] = psp.tile(
                        [HD + 1, 512], FP32, name=f"pso{hp}{hh}{qf}",
                        tag="ps_acc", bufs=4)

            def emit_o(hp, m, qf):
                vd3 = vd_sb[m][:].rearrange("p (h e) -> p h e", e=HD + 1)
                pu = pu_tiles[(hp, m, qf)]
                for hh in range(2):
                    nc.tensor.matmul(
                        ps_o[(hp, hh, qf)][:],
                        lhsT=vd3[:, 2 * hp + hh, :],
                        rhs=pu[:, hh * 512:(hh + 1) * 512],
                        start=(m == 0), stop=(m == NKT - 1))

            def emit_norm(hp, qf):
                # o_sb[hp][0:64, qf half] = head 2hp, [64:128] = head 2hp+1,
                # each row block scaled by its softmax reciprocal. The
                # reciprocal reads the accumulator's ones-row straight from
                # PSUM; Pool broadcasts it across the 64 head partitions.
                cols = slice(qf * 512, (qf + 1) * 512)
                recs = []
                for hh in range(2):
                    dn = sm.tile([1, 512], FP32, name=f"dn{hp}{hh}{qf}",
                                 tag="dn", bufs=4)
                    nc.vector.tensor_copy(dn[:],
                                          ps_o[(hp, hh, qf)][HD:HD + 1, :])
                    rec = sm.tile([1, 512], FP32, name=f"rec{hp}{hh}{qf}",
                                  tag="rec", bufs=4)
                    nc.vector.reciprocal_approx_fast(rec[:], dn[:])
                    recs.append(rec)
                for hh in range(2):
                    bc = sm.tile([HD, 512], FP32, name=f"bc{hp}{hh}{qf}",
                                 tag="bc", bufs=2)
                    nc.gpsimd.partition_broadcast(bc[:], recs[hh][:])
                    nc.vector.tensor_mul(o_sb[hp][hh * HD:(hh + 1) * HD, cols],
                                         ps_o[(hp, hh, qf)][0:HD, :],
                                         bc[:])

            def emit_wo(qf, pts=range(CT)):
                cols = slice(qf * 512, (qf + 1) * 512)
                for pt in pts:
                    ps = psp.tile([P, 512], FP32, name=f"psw{pt}{qf}",
                                  tag="ps_acc", bufs=4)
                    for hp in range(G):
                        nc.tensor.matmul(
                            ps[:],
                            lhsT=wo_sb[:, hp * C + pt * P:
                                       hp * C + (pt + 1) * P],
                            rhs=o_sb[hp][:, cols],
                            start=(hp == 0), stop=(hp == G - 1))
                    ob = sm.tile([P, 512], FP16, name=f"ob{pt}{qf}",
                                 tag="ob", bufs=2)
                    nc.scalar.copy(ob[:], ps[:])
                    nc.sync.dma_start(out=out_d[pt * P:(pt + 1) * P, cols],
                                      in_=ob[:])

            # ---- emission schedule ----------------------------------------
            # Phase A: minimal prologue for the first S pair.
            emit_qt_chunk(0, 0)
            emit_kt_chunk(0, 0)

            # Phase B: hp0 spine (qf-outer: all qf0 units then all qf1), PE
            # backfill ordered by DMA arrival and need-by unit: KT(hp0) JIT
            # chunks, QT(hp0,qf1) before unit 8, QT/KT(hp1), V, Vd.
            backfill = ([lambda j=j: emit_kt_chunk(0, j) for j in (1, 2, 3)]
                        + [lambda: emit_qt_chunk(0, 1),
                           lambda: emit_qt_chunk(1, 0),
                           lambda: emit_qt_chunk(1, 1)]
                        + [lambda j=j: emit_kt_chunk(1, j) for j in range(4)]
                        + [lambda m=m: emit_v(m) for m in range(NKT)]
                        + [lambda m=m: emit_vd(m) for m in range(NKT)])
            # backfill items to run after spine unit u (16 units, 26 items):
            # KT0 JIT (c1 by unit 2, c2 by 4, c3 by 6), QT0qf1 by unit 8,
            # V from unit ~5 (wv lands ~6us), Vd from unit ~9 (gt ~10us).
            steps = [1, 1, 1, 1, 2, 2, 2, 2, 2, 2, 2, 2, 2, 2, 1, 1]
            bi = 0
            for u, (qf, m) in enumerate([(qf, m) for qf in range(2)
                                         for m in range(NKT)]):
                emit_s(0, m, qf)
                for _ in range(steps[u]):
                    if bi < len(backfill):
                        backfill[bi]()
                        bi += 1
            while bi < len(backfill):
                backfill[bi]()
                bi += 1

            # Phase C: hp1 spine (qf-outer). PE backfill: O(hp0) qf-grouped
            # then norm(hp0); O(hp1) lags once its accumulators are free.
            c_spine = [(qf, m) for qf in range(2) for m in range(NKT)]
            # O(hp0): all 16 (m, qf) pairs, qf-grouped, 2 per spine unit.
            alloc_ps_o(0, 0)
            alloc_ps_o(0, 1)
            o0_fill = ([(0, m, 0) for m in range(NKT)]
                       + [(0, m, 1) for m in range(NKT)])
            o0i = 0
            o1_ready = []          # (hp1, m, qf) O units whose pu exists
            o1i = 0
            norm0_done = False
            o1_alloc = [False, False]

            def drain_o1(limit):
                nonlocal o1i
                while o1i < len(o1_ready) and o1i < limit:
                    m, qf = o1_ready[o1i]
                    if not o1_alloc[qf]:
                        alloc_ps_o(1, qf)
                        o1_alloc[qf] = True
                    emit_o(1, m, qf)
                    o1i += 1

            norm1q0_done = False
            for u, (qf, m) in enumerate(c_spine):
                emit_s(1, m, qf)
                o1_ready.append((m, qf))
                # backfill after this spine unit:
                if o0i < len(o0_fill):
                    # O(hp0): 2 units per spine unit, norm(0) after the last
                    for _ in range(2):
                        if o0i < len(o0_fill):
                            emit_o(*o0_fill[o0i])
                            o0i += 1
                    if o0i == len(o0_fill) and not norm0_done:
                        emit_norm(0, 0)
                        emit_norm(0, 1)
                        norm0_done = True
                else:
                    # O(hp1) streams with a lag >= 2 units behind the spine,
                    # paced to avoid bursts that would stall the exp stream
                    # (first drain catches up by 3, then 2 per unit).
                    drain_o1(min(len(o1_ready) - 2,
                                 o1i + (3 if o1i == 0 else 2)))
                if o1i >= 8 and not norm1q0_done:
                    emit_norm(1, 0)    # qf0 accumulators complete
                    norm1q0_done = True

            # Phase D: tail. Remaining O(hp1,qf1), Wo(qf0) under norm(1,qf1),
            # then Wo(qf1), per-pt pipelined fp16 copies + out DMA.
            drain_o1(len(o1_ready) - 1)
            emit_wo(0, pts=(0, 1))
            drain_o1(len(o1_ready))
            emit_norm(1, 1)
            emit_wo(0, pts=(2, 3))
            emit_wo(1)

    nc.compile()
    return nc


# --------------------------------------------------------------------------
# public entry points
# --------------------------------------------------------------------------

def _prepare(inputs):
    q = np.ascontiguousarray(inputs["q_feat"], np.float32).reshape(B, C, N)
    kv = np.ascontiguousarray(inputs["kv_feat"], np.float32).reshape(B, C, N)
    offsets = np.asarray(inputs["offsets"], np.float32)
    sal = np.asarray(inputs["saliency_map"], np.float32).reshape(B, N)
    gamma = float(np.asarray(inputs["gamma_val"]))

    GTs = [_gather_T(offsets[b], 1.0 + gamma * sal[b]) for b in range(B)]

    # union band-sparsity pattern of the gather matmul across batches, so the
    # SPMD program is identical on every core
    chunks = []
    for m in range(NKT):
        for k in range(NKT):
            if any(GTs[b][k * P:(k + 1) * P, m * P:(m + 1) * P].any()
                   for b in range(B)):
                chunks.append((m, k))

    Wq = np.asarray(inputs["Wq"], np.float32) * (HD ** -0.5)
    Wk = np.asarray(inputs["Wk"], np.float32)
    Wv = np.asarray(inputs["Wv"], np.float32)
    Wo = np.asarray(inputs["Wo"], np.float32)

    def ptile(a):
        # [T*P, X] -> [P, T*X]: partition-tile-major columns
        t = a.shape[0] // P
        return np.ascontiguousarray(
            a.reshape(t, P, a.shape[1]).transpose(1, 0, 2).reshape(P, -1)
        ).astype(NP_BF16)

    in_maps = []
    for core in range(NCORES):
        b, g = core // G, core % G
        rows = slice(g * CG, (g + 1) * CG)
        gt_stack = np.stack([GTs[b][k * P:(k + 1) * P, m * P:(m + 1) * P]
                             for (m, k) in chunks])     # [nch, P, P]
        in_maps.append({
            "xq": ptile(q[b]),
            "xkv": ptile(kv[b]),
            "wqT": ptile(Wq[rows].T),
            "wkT": ptile(Wk[rows].T),
            "wvT": ptile(Wv[rows].T),
            "woT": ptile(Wo[:, rows].T),
            "gt": np.ascontiguousarray(
                gt_stack.transpose(1, 0, 2).reshape(P, -1)).astype(NP_BF16),
        })

    def assemble(results):
        out = np.empty((B, C, N), np.float32)
        for b in range(B):
            out[b] = (results[G * b]["outT"].astype(np.float32)
                      + results[G * b + 1]["outT"].astype(np.float32))
        return out.reshape(B, C, HH, WW)

    nc = _build_program(chunks)
    return nc, in_maps, assemble


def _needs_fallback(inputs):
    try:
        if tuple(np.shape(inputs["q_feat"])) != (B, C, HH, WW):
            return True
        for bias in ("bq", "bk", "bv", "bo"):
            if np.any(np.asarray(inputs[bias], np.float32) != 0.0):
                return True
    except Exception:
        return True
    return False


def kernel(**inputs) -> np.ndarray:
    if _needs_fallback(inputs):
        return _reference_numpy(**{k: np.asarray(v, np.float32)
                                   for k, v in inputs.items()})
    nc, in_maps, assemble = _prepare(inputs)
    res = run_bass_kernel_spmd(nc, in_maps, core_ids=list(range(NCORES)))
    return assemble(res.results)


def kernel_traced(trace_cores=(0,), **inputs):
    """Like kernel() but returns (output, exec_time_ns, trace_path)."""
    nc, in_maps, assemble = _prepare(inputs)
    res = run_bass_kernel_spmd(nc, in_maps, core_ids=list(range(NCORES)),
                               trace=True, trace_cores=list(trace_cores))
    trace_path = None
    if res.instructions_and_trace is not None:
        trace_path = res.instructions_and_trace[1]
    return assemble(res.results), res.exec_time_ns, trace_path
